# revision 1
# baseline (speedup 1.0000x reference)
"""Trainium2 Bass kernel for GaussianFPSPooling.

Pipeline (per batch element, one NeuronCore):
  1. Farthest-point sampling over N=100000 3-D points, K=256 iterations,
     fully SBUF-resident.  Arithmetic replicates the jax-CPU reference
     bit-exactly ((x-px)^2 + (y-py)^2) + (z-pz)^2, f32, left-assoc, min
     accumulate, first-index argmax) so the selected indices match.
  2. Indirect-DMA gather of the 256 selected feature rows from HBM.
  3. PE transpose + matmul with W (f32) + bias.

Distribution: data-parallel over the batch (B=4) across 8 cores; cores
c and c+4 run the same batch (c % 4), host reads cores 0-3.
"""

import sys

if "/opt/trn_rl_repo" not in sys.path:
    sys.path.insert(0, "/opt/trn_rl_repo")

import numpy as np

import concourse.bacc as bacc
import concourse.bass as bass
import concourse.bass_isa as bass_isa
import concourse.mybir as mybir
from concourse import tile
from concourse.bass_utils import run_bass_kernel_spmd

F32 = mybir.dt.float32
I32 = mybir.dt.int32
Alu = mybir.AluOpType
Act = mybir.ActivationFunctionType

# problem sizes (hardcoded per contract)
B = 4
N = 100000
D_IN = 128
D_OUT = 256
K = 256
P = 128               # partitions
BIGI = float(1 << 20)  # index-encoding base: stores BIGI - idx (exact in f32)


def _ceil_div(a, b):
    return (a + b - 1) // b


def build_fps_kernel(n=N, k=K, d_in=D_IN, d_out=D_OUT, with_linear=True):
    """Build the Bass program. Returns (nc, C) with C = cols per partition."""
    C = _ceil_div(n, P)
    npad = P * C

    nc = bacc.Bacc("TRN2", target_bir_lowering=False)

    # ---- DRAM I/O ----
    xs_d = nc.dram_tensor("xs", [P, C], F32, kind="ExternalInput")
    ys_d = nc.dram_tensor("ys", [P, C], F32, kind="ExternalInput")
    zs_d = nc.dram_tensor("zs", [P, C], F32, kind="ExternalInput")
    g2_d = nc.dram_tensor("g2", [P, C], F32, kind="ExternalInput")
    dists_d = nc.dram_tensor("dists0", [P, C], F32, kind="ExternalInput")
    pt0_d = nc.dram_tensor("pt0", [P, 4], F32, kind="ExternalInput")
    idx_d = nc.dram_tensor("idx_out", [1, k], F32, kind="ExternalOutput")
    if with_linear:
        feat_d = nc.dram_tensor("feat", [n, d_in], F32, kind="ExternalInput")
        w_d = nc.dram_tensor("w", [d_in, d_out], F32, kind="ExternalInput")
        brow_d = nc.dram_tensor("brow", [1, d_out], F32, kind="ExternalInput")
        ones1_d = nc.dram_tensor("ones1", [1, P], F32, kind="ExternalInput")
        ident_d = nc.dram_tensor("ident", [P, P], F32, kind="ExternalInput")
        out_d = nc.dram_tensor("out", [k, d_out], F32, kind="ExternalOutput")

    kg = k // P if with_linear else 0  # gather column groups
    if with_linear:
        assert k % P == 0

    # position of iteration-k index inside idxraw (so a plain [1,k]->[P,kg]
    # SBUF->SBUF DMA lands index of sample k at partition k%P, col k//P)
    if with_linear:
        pos = [(kk % P) * kg + (kk // P) for kk in range(k)]
    else:
        pos = list(range(k))

    with tile.TileContext(nc) as tc:
        with (
            tc.tile_pool(name="const", bufs=1) as cp,
            tc.tile_pool(name="loop", bufs=2) as lp,
            tc.tile_pool(name="psum", bufs=2, space="PSUM") as pp,
        ):
            xs = cp.tile([P, C], F32, tag="xs")
            ys = cp.tile([P, C], F32, tag="ys")
            zs = cp.tile([P, C], F32, tag="zs")
            g2 = cp.tile([P, C], F32, tag="g2")
            dists = cp.tile([P, C], F32, tag="dists")
            pt0 = cp.tile([P, 4], F32, tag="pt0")
            idxraw = cp.tile([1, k], F32, tag="idxraw")

            nc.sync.dma_start(xs[:], xs_d[:])
            nc.sync.dma_start(ys[:], ys_d[:])
            nc.sync.dma_start(zs[:], zs_d[:])
            nc.sync.dma_start(g2[:], g2_d[:])
            nc.sync.dma_start(dists[:], dists_d[:])
            nc.sync.dma_start(pt0[:], pt0_d[:])
            nc.vector.memset(idxraw[:], BIGI)  # sample 0 is point 0

            pt = pt0
            for it in range(k - 1):
                px = pt[:, 0:1]
                py = pt[:, 1:2]
                pz = pt[:, 2:3]
                # d = ((x-px)^2 + (y-py)^2) + (z-pz)^2, bit-exact f32
                t1 = lp.tile([P, C], F32, tag="t1")
                nc.scalar.activation(t1[:], xs[:], Act.Square, bias=px, scale=-1.0)
                t2 = lp.tile([P, C], F32, tag="t2")
                nc.scalar.activation(t2[:], ys[:], Act.Square, bias=py, scale=-1.0)
                t3 = lp.tile([P, C], F32, tag="t3")
                nc.scalar.activation(t3[:], zs[:], Act.Square, bias=pz, scale=-1.0)
                s = lp.tile([P, C], F32, tag="s")
                nc.vector.tensor_tensor(s[:], t1[:], t2[:], op=Alu.add)
                nc.vector.tensor_tensor(s[:], s[:], t3[:], op=Alu.add)
                # dists = min(dists, d); permax = rowwise max of new dists
                # (tensor_tensor_reduce would fuse these but crashes this
                # runtime, so keep them split)
                permax = lp.tile([P, 1], F32, tag="permax")
                nc.vector.tensor_tensor(dists[:], dists[:], s[:], op=Alu.min)
                nc.vector.reduce_max(permax[:], dists[:], axis=mybir.AxisListType.X)
                gmax = lp.tile([P, 1], F32, tag="gmax")
                nc.gpsimd.partition_all_reduce(
                    gmax[:], permax[:], channels=P, reduce_op=bass_isa.ReduceOp.max
                )
                # encode argmax as max over (dists==gmax)*(BIGI-idx)
                mi = lp.tile([P, C], F32, tag="mi")
                nc.vector.scalar_tensor_tensor(
                    mi[:], in0=dists[:], scalar=gmax[:], in1=g2[:],
                    op0=Alu.is_equal, op1=Alu.mult,
                )
                permax2 = lp.tile([P, 1], F32, tag="permax2")
                nc.vector.reduce_max(permax2[:], mi[:], axis=mybir.AxisListType.X)
                is2 = lp.tile([P, 1], F32, tag="is2")
                nc.gpsimd.partition_all_reduce(
                    is2[:], permax2[:], channels=P, reduce_op=bass_isa.ReduceOp.max
                )
                # record BIGI - idx (decoded after the loop)
                nc.scalar.copy(idxraw[0:1, pos[it + 1] : pos[it + 1] + 1],
                               is2[0:1, 0:1])
                # extract winner coords: one-hot (g2==is2) dot each plane
                ptn = lp.tile([P, 4], F32, tag="ptn")
                junk = lp.tile([P, C], F32, tag="junk")
                nc.vector.scalar_tensor_tensor(
                    junk[:], in0=g2[:], scalar=is2[:], in1=xs[:],
                    op0=Alu.is_equal, op1=Alu.mult, accum_out=ptn[:, 0:1],
                )
                nc.vector.scalar_tensor_tensor(
                    junk[:], in0=g2[:], scalar=is2[:], in1=ys[:],
                    op0=Alu.is_equal, op1=Alu.mult, accum_out=ptn[:, 1:2],
                )
                nc.vector.scalar_tensor_tensor(
                    junk[:], in0=g2[:], scalar=is2[:], in1=zs[:],
                    op0=Alu.is_equal, op1=Alu.mult, accum_out=ptn[:, 2:3],
                )
                ptb = lp.tile([P, 4], F32, tag="ptb")
                nc.gpsimd.partition_all_reduce(
                    ptb[:, 0:3], ptn[:, 0:3], channels=P,
                    reduce_op=bass_isa.ReduceOp.add,
                )
                pt = ptb

            # decode indices: idx = BIGI - idxraw
            idxf = cp.tile([1, k], F32, tag="idxf")
            nc.vector.tensor_scalar(
                idxf[:], idxraw[:], -1.0, BIGI, op0=Alu.mult, op1=Alu.add
            )
            nc.sync.dma_start(idx_d[:], idxf[:])

            if with_linear:
                w_sb = cp.tile([d_in, d_out], F32, tag="w")
                brow = cp.tile([1, d_out], F32, tag="brow")
                ones1 = cp.tile([1, P], F32, tag="ones1")
                ident = cp.tile([P, P], F32, tag="ident")
                nc.sync.dma_start(w_sb[:], w_d[:])
                nc.sync.dma_start(brow[:], brow_d[:])
                nc.sync.dma_start(ones1[:], ones1_d[:])
                nc.sync.dma_start(ident[:], ident_d[:])

                idxi = cp.tile([1, k], I32, tag="idxi")
                nc.vector.tensor_copy(idxi[:], idxf[:])
                gidx = cp.tile([P, kg], I32, tag="gidx")
                nc.sync.dma_start(gidx[:], idxi[:])  # relayout [1,k]->[P,kg]

                for j in range(kg):
                    gath = cp.tile([P, d_in], F32, tag=f"gath{j}")
                    nc.gpsimd.indirect_dma_start(
                        out=gath[:],
                        out_offset=None,
                        in_=feat_d[:],
                        in_offset=bass.IndirectOffsetOnAxis(
                            ap=gidx[:, j : j + 1], axis=0
                        ),
                    )
                    tp_ps = pp.tile([P, P], F32, tag="tp")
                    nc.tensor.transpose(tp_ps[:], gath[:], ident[:])
                    lhsT = cp.tile([P, P], F32, tag=f"lhsT{j}")
                    nc.vector.tensor_copy(lhsT[:], tp_ps[:])
                    out_ps = pp.tile([P, d_out], F32, tag="outps")
                    nc.tensor.matmul(
                        out_ps[:], lhsT=lhsT[:], rhs=w_sb[:], start=True, stop=False
                    )
                    nc.tensor.matmul(
                        out_ps[:], lhsT=ones1[:], rhs=brow[:], start=False, stop=True
                    )
                    outt = cp.tile([P, d_out], F32, tag=f"outt{j}")
                    nc.vector.tensor_copy(outt[:], out_ps[:])
                    nc.sync.dma_start(out_d[j * P : (j + 1) * P, :], outt[:])

    nc.compile()
    return nc, C


def make_core_inputs(means_b, features_b=None, W=None, bvec=None,
                     n=N, k=K, with_linear=True):
    """Host-side layout for one batch element."""
    C = _ceil_div(n, P)
    npad = P * C
    m = np.asarray(means_b, np.float32)
    planes = np.zeros((npad, 3), np.float32)
    planes[:n] = m
    d0 = np.full(npad, -1.0, np.float32)
    d0[:n] = np.inf
    g2 = np.zeros(npad, np.float32)
    g2[:n] = BIGI - np.arange(n, dtype=np.float32)
    pt0 = np.zeros((P, 4), np.float32)
    pt0[:, 0:3] = m[0]
    d = {
        "xs": planes[:, 0].reshape(P, C).copy(),
        "ys": planes[:, 1].reshape(P, C).copy(),
        "zs": planes[:, 2].reshape(P, C).copy(),
        "g2": g2.reshape(P, C).copy(),
        "dists0": d0.reshape(P, C).copy(),
        "pt0": pt0,
    }
    if with_linear:
        d["feat"] = np.ascontiguousarray(features_b, dtype=np.float32)
        d["w"] = np.ascontiguousarray(W, dtype=np.float32)
        d["brow"] = np.ascontiguousarray(bvec, dtype=np.float32).reshape(1, -1)
        d["ones1"] = np.ones((1, P), np.float32)
        d["ident"] = np.eye(P, dtype=np.float32)
    return d


_CACHE = {}


def _get_kernel():
    if "nc" not in _CACHE:
        _CACHE["nc"] = build_fps_kernel()[0]
    return _CACHE["nc"]


def kernel(features, means, W, b, trace=False):
    features = np.asarray(features, np.float32)
    means = np.asarray(means, np.float32)
    W = np.asarray(W, np.float32)
    b = np.asarray(b, np.float32)

    nc = _get_kernel()
    in_maps = []
    for c in range(8):
        bb = c % B
        in_maps.append(make_core_inputs(means[bb], features[bb], W, b))
    import time as _time

    t0 = _time.time()
    res = run_bass_kernel_spmd(nc, in_maps, core_ids=list(range(8)), trace=trace)
    _CACHE["last_run_s"] = _time.time() - t0
    out = np.stack([res.results[bb]["out"] for bb in range(B)], axis=0)
    _CACHE["last_results"] = res
    return out


if __name__ == "__main__":
    ins = dict(np.load("/tmp/inputs.npz"))
    out = kernel(**ins)
    print("out", out.shape, out.dtype)



# revision 2
# speedup vs baseline: 7.9922x; 7.9922x over previous
"""Trainium2 Bass kernel for GaussianFPSPooling.

The axon tunnel to the device moves ~40 MB/s, so the run is dominated by
host<->device transfer, not compute.  The old baseline shipped the full
features tensor (8 x 51.2 MB) every call.  This version never sends
features to the device:

  Phase 1 (device, cores 0-3, one batch element per core):
      Farthest-point sampling over N=100000 3-D points, K=256 iterations,
      fully SBUF-resident.  Arithmetic replicates the jax-CPU reference
      bit-exactly ((x-px)^2 + (y-py)^2) + (z-pz)^2, f32, left-assoc, min
      accumulate, first-index argmax) so the selected indices match.
      Only the coordinate planes (1.2 MB/batch) go over the wire; the
      index-encoding plane and the init-distance plane are generated
      on-device with iota.  Returns the K indices (1 KB/core).

  Host: gather the K=256 selected feature rows per batch (numpy fancy
      indexing, microseconds) and transpose to [d_in, K].

  Phase 2 (device, cores 0-3): rowsT.T @ W + b per batch via two
      PE matmuls (+ ones x b trick for the bias).  Ships 257 KB/core in,
      256 KB/core out.

Total wire traffic ~7 MB/call instead of ~410 MB.
"""

import sys

if "/opt/trn_rl_repo" not in sys.path:
    sys.path.insert(0, "/opt/trn_rl_repo")

import numpy as np

import concourse.bacc as bacc
import concourse.bass as bass
import concourse.bass_isa as bass_isa
import concourse.mybir as mybir
from concourse import tile
from concourse.bass_utils import run_bass_kernel_spmd

F32 = mybir.dt.float32
I32 = mybir.dt.int32
Alu = mybir.AluOpType
Act = mybir.ActivationFunctionType

# problem sizes (hardcoded per contract)
B = 4
N = 100000
D_IN = 128
D_OUT = 256
K = 256
P = 128               # partitions
BIGI = float(1 << 20)  # index-encoding base: stores BIGI - idx (exact in f32)
BIG = 1.0e30          # init "infinity" for valid entries; pad gets -BIG


def _ceil_div(a, b):
    return (a + b - 1) // b


def build_fps_kernel(n=N, k=K):
    """Phase-1 program: FPS over one batch element; emits the K indices."""
    C = _ceil_div(n, P)

    nc = bacc.Bacc("TRN2", target_bir_lowering=False)

    xs_d = nc.dram_tensor("xs", [P, C], F32, kind="ExternalInput")
    ys_d = nc.dram_tensor("ys", [P, C], F32, kind="ExternalInput")
    zs_d = nc.dram_tensor("zs", [P, C], F32, kind="ExternalInput")
    pt0_d = nc.dram_tensor("pt0", [P, 4], F32, kind="ExternalInput")
    idx_d = nc.dram_tensor("idx_out", [1, k], F32, kind="ExternalOutput")

    with tile.TileContext(nc) as tc:
        with (
            tc.tile_pool(name="const", bufs=1) as cp,
            tc.tile_pool(name="loop", bufs=2) as lp,
        ):
            xs = cp.tile([P, C], F32, tag="xs")
            ys = cp.tile([P, C], F32, tag="ys")
            zs = cp.tile([P, C], F32, tag="zs")
            g2 = cp.tile([P, C], F32, tag="g2")
            dists = cp.tile([P, C], F32, tag="dists")
            pt0 = cp.tile([P, 4], F32, tag="pt0")
            idxraw = cp.tile([1, k], F32, tag="idxraw")

            nc.sync.dma_start(xs[:], xs_d[:])
            nc.sync.dma_start(ys[:], ys_d[:])
            nc.sync.dma_start(zs[:], zs_d[:])
            nc.sync.dma_start(pt0[:], pt0_d[:])
            nc.vector.memset(idxraw[:], BIGI)  # sample 0 is point 0

            # generate flat point index p*C + c on device, then derive the
            # argmax-encoding plane g2 = BIGI - idx and the init distances
            # (+BIG valid / -BIG pad) from it
            ii = cp.tile([P, C], I32, tag="ii")
            nc.gpsimd.iota(ii[:], [[1, C]], channel_multiplier=C)
            idxpc = cp.tile([P, C], F32, tag="idxpc")
            nc.vector.tensor_copy(idxpc[:], ii[:])
            nc.vector.tensor_scalar(
                g2[:], idxpc[:], -1.0, BIGI, op0=Alu.mult, op1=Alu.add
            )
            valid = cp.tile([P, C], F32, tag="valid")
            nc.vector.tensor_scalar(valid[:], idxpc[:], float(n), None, op0=Alu.is_lt)
            nc.vector.tensor_scalar(
                dists[:], valid[:], 2.0 * BIG, -BIG, op0=Alu.mult, op1=Alu.add
            )

            pt = pt0
            for it in range(k - 1):
                px = pt[:, 0:1]
                py = pt[:, 1:2]
                pz = pt[:, 2:3]
                # d = ((x-px)^2 + (y-py)^2) + (z-pz)^2, bit-exact f32
                t1 = lp.tile([P, C], F32, tag="t1")
                nc.scalar.activation(t1[:], xs[:], Act.Square, bias=px, scale=-1.0)
                t2 = lp.tile([P, C], F32, tag="t2")
                nc.scalar.activation(t2[:], ys[:], Act.Square, bias=py, scale=-1.0)
                t3 = lp.tile([P, C], F32, tag="t3")
                nc.scalar.activation(t3[:], zs[:], Act.Square, bias=pz, scale=-1.0)
                s = lp.tile([P, C], F32, tag="s")
                nc.vector.tensor_tensor(s[:], t1[:], t2[:], op=Alu.add)
                nc.vector.tensor_tensor(s[:], s[:], t3[:], op=Alu.add)
                # dists = min(dists, d); permax = rowwise max of new dists
                # (tensor_tensor_reduce would fuse these but crashes this
                # runtime, so keep them split)
                permax = lp.tile([P, 1], F32, tag="permax")
                nc.vector.tensor_tensor(dists[:], dists[:], s[:], op=Alu.min)
                nc.vector.reduce_max(permax[:], dists[:], axis=mybir.AxisListType.X)
                gmax = lp.tile([P, 1], F32, tag="gmax")
                nc.gpsimd.partition_all_reduce(
                    gmax[:], permax[:], channels=P, reduce_op=bass_isa.ReduceOp.max
                )
                # encode argmax as max over (dists==gmax)*(BIGI-idx)
                mi = lp.tile([P, C], F32, tag="mi")
                nc.vector.scalar_tensor_tensor(
                    mi[:], in0=dists[:], scalar=gmax[:], in1=g2[:],
                    op0=Alu.is_equal, op1=Alu.mult,
                )
                permax2 = lp.tile([P, 1], F32, tag="permax2")
                nc.vector.reduce_max(permax2[:], mi[:], axis=mybir.AxisListType.X)
                is2 = lp.tile([P, 1], F32, tag="is2")
                nc.gpsimd.partition_all_reduce(
                    is2[:], permax2[:], channels=P, reduce_op=bass_isa.ReduceOp.max
                )
                # record BIGI - idx (decoded after the loop)
                nc.scalar.copy(idxraw[0:1, it + 1 : it + 2], is2[0:1, 0:1])
                # extract winner coords: one-hot (g2==is2) dot each plane
                ptn = lp.tile([P, 4], F32, tag="ptn")
                junk = lp.tile([P, C], F32, tag="junk")
                nc.vector.scalar_tensor_tensor(
                    junk[:], in0=g2[:], scalar=is2[:], in1=xs[:],
                    op0=Alu.is_equal, op1=Alu.mult, accum_out=ptn[:, 0:1],
                )
                nc.vector.scalar_tensor_tensor(
                    junk[:], in0=g2[:], scalar=is2[:], in1=ys[:],
                    op0=Alu.is_equal, op1=Alu.mult, accum_out=ptn[:, 1:2],
                )
                nc.vector.scalar_tensor_tensor(
                    junk[:], in0=g2[:], scalar=is2[:], in1=zs[:],
                    op0=Alu.is_equal, op1=Alu.mult, accum_out=ptn[:, 2:3],
                )
                ptb = lp.tile([P, 4], F32, tag="ptb")
                nc.gpsimd.partition_all_reduce(
                    ptb[:, 0:3], ptn[:, 0:3], channels=P,
                    reduce_op=bass_isa.ReduceOp.add,
                )
                pt = ptb

            # decode indices: idx = BIGI - idxraw
            idxf = cp.tile([1, k], F32, tag="idxf")
            nc.vector.tensor_scalar(
                idxf[:], idxraw[:], -1.0, BIGI, op0=Alu.mult, op1=Alu.add
            )
            nc.sync.dma_start(idx_d[:], idxf[:])

    nc.compile()
    return nc


def build_linear_kernel(k=K, d_in=D_IN, d_out=D_OUT):
    """Phase-2 program: out = rowsT.T @ W + b for one batch element."""
    assert k % P == 0 and d_in == P
    kg = k // P

    nc = bacc.Bacc("TRN2", target_bir_lowering=False)

    rowsT_d = nc.dram_tensor("rowsT", [d_in, k], F32, kind="ExternalInput")
    w_d = nc.dram_tensor("w", [d_in, d_out], F32, kind="ExternalInput")
    brow_d = nc.dram_tensor("brow", [1, d_out], F32, kind="ExternalInput")
    out_d = nc.dram_tensor("out", [k, d_out], F32, kind="ExternalOutput")

    with tile.TileContext(nc) as tc:
        with (
            tc.tile_pool(name="const", bufs=1) as cp,
            tc.tile_pool(name="psum", bufs=2, space="PSUM") as pp,
        ):
            rowsT = cp.tile([d_in, k], F32, tag="rowsT")
            w_sb = cp.tile([d_in, d_out], F32, tag="w")
            brow = cp.tile([1, d_out], F32, tag="brow")
            ones1 = cp.tile([1, P], F32, tag="ones1")
            nc.sync.dma_start(rowsT[:], rowsT_d[:])
            nc.sync.dma_start(w_sb[:], w_d[:])
            nc.sync.dma_start(brow[:], brow_d[:])
            nc.vector.memset(ones1[:], 1.0)

            for j in range(kg):
                out_ps = pp.tile([P, d_out], F32, tag="outps")
                nc.tensor.matmul(
                    out_ps[:], lhsT=rowsT[:, j * P : (j + 1) * P], rhs=w_sb[:],
                    start=True, stop=False,
                )
                nc.tensor.matmul(
                    out_ps[:], lhsT=ones1[:], rhs=brow[:], start=False, stop=True
                )
                outt = cp.tile([P, d_out], F32, tag=f"outt{j}")
                nc.vector.tensor_copy(outt[:], out_ps[:])
                nc.sync.dma_start(out_d[j * P : (j + 1) * P, :], outt[:])

    nc.compile()
    return nc


def make_fps_inputs(means_b, n=N):
    """Host-side layout of one batch element's coordinate planes."""
    C = _ceil_div(n, P)
    npad = P * C
    m = np.asarray(means_b, np.float32)
    planes = np.zeros((npad, 3), np.float32)
    planes[:n] = m
    pt0 = np.zeros((P, 4), np.float32)
    pt0[:, 0:3] = m[0]
    return {
        "xs": planes[:, 0].reshape(P, C).copy(),
        "ys": planes[:, 1].reshape(P, C).copy(),
        "zs": planes[:, 2].reshape(P, C).copy(),
        "pt0": pt0,
    }


_CACHE = {}


def _get_kernels():
    if "nc_fps" not in _CACHE:
        _CACHE["nc_fps"] = build_fps_kernel()
        _CACHE["nc_lin"] = build_linear_kernel()
    return _CACHE["nc_fps"], _CACHE["nc_lin"]


def kernel(features, means, W, b, trace=False):
    features = np.asarray(features, np.float32)
    means = np.asarray(means, np.float32)
    W = np.ascontiguousarray(W, np.float32)
    brow = np.ascontiguousarray(b, np.float32).reshape(1, -1)

    nc_fps, nc_lin = _get_kernels()
    import time as _time

    t0 = _time.time()
    fps_maps = [make_fps_inputs(means[bb]) for bb in range(B)]
    res1 = run_bass_kernel_spmd(nc_fps, fps_maps, core_ids=list(range(B)),
                                trace=trace)
    idx = np.stack(
        [np.rint(res1.results[bb]["idx_out"][0]).astype(np.int64)
         for bb in range(B)]
    )  # [B, K]

    lin_maps = [
        {
            "rowsT": np.ascontiguousarray(features[bb][idx[bb]].T),
            "w": W,
            "brow": brow,
        }
        for bb in range(B)
    ]
    res2 = run_bass_kernel_spmd(nc_lin, lin_maps, core_ids=list(range(B)),
                                trace=trace)
    _CACHE["last_run_s"] = _time.time() - t0
    out = np.stack([res2.results[bb]["out"] for bb in range(B)], axis=0)
    _CACHE["last_results"] = res2
    return out


if __name__ == "__main__":
    ins = dict(np.load("/tmp/inputs.npz"))
    out = kernel(**ins)
    print("out", out.shape, out.dtype)


# revision 24
# speedup vs baseline: 55.7118x; 6.9707x over previous
"""Trainium2 Bass kernel for GaussianFPSPooling.

The axon tunnel to the device moves ~40 MB/s, so the run is dominated by
host<->device transfer, not compute.  The old baseline shipped the full
features tensor (8 x 51.2 MB) every call.  This version never sends
features to the device:

  Phase 1 (device, cores 0-3, one batch element per core):
      Farthest-point sampling over N=100000 3-D points, K=256 iterations,
      fully SBUF-resident.  Arithmetic replicates the jax-CPU reference
      bit-exactly ((x-px)^2 + (y-py)^2) + (z-pz)^2, f32, left-assoc, min
      accumulate, first-index argmax) so the selected indices match.
      Only the coordinate planes (1.2 MB/batch) go over the wire; the
      index-encoding plane and the init-distance plane are generated
      on-device with iota.  Returns the K indices (1 KB/core).

  Host: gather the K=256 selected feature rows per batch (numpy fancy
      indexing, microseconds) and transpose to [d_in, K].

  Phase 2 (device, cores 0-3): rowsT.T @ W + b per batch via two
      PE matmuls (+ ones x b trick for the bias).  Ships 257 KB/core in,
      256 KB/core out.

Total wire traffic ~7 MB/call instead of ~410 MB.
"""

import sys

if "/opt/trn_rl_repo" not in sys.path:
    sys.path.insert(0, "/opt/trn_rl_repo")

import numpy as np

import concourse.bacc as bacc
import concourse.bass as bass
import concourse.bass_isa as bass_isa
import concourse.mybir as mybir
from concourse import tile
from concourse.bass_utils import run_bass_kernel_spmd

F32 = mybir.dt.float32
I32 = mybir.dt.int32
Alu = mybir.AluOpType
Act = mybir.ActivationFunctionType

# problem sizes (hardcoded per contract)
B = 4
N = 100000
D_IN = 128
D_OUT = 256
K = 256
P = 128               # partitions
BIGI = float(1 << 20)  # index-encoding base: stores BIGI - idx (exact in f32)
BIG = 1.0e30          # init "infinity" for valid entries; pad gets -BIG


def _ceil_div(a, b):
    return (a + b - 1) // b


def build_fps_kernel(n=N, k=K):
    """Phase-1 program: FPS over one batch element; emits the K indices."""
    C = _ceil_div(n, P)

    nc = bacc.Bacc("TRN2", target_bir_lowering=False)

    # all coordinate planes + the seed point packed into ONE input tensor:
    # a single transfer RPC over the (high-latency) axon link
    xyzp_d = nc.dram_tensor("xyzp", [P, 3 * C + 4], F32, kind="ExternalInput")
    idx_d = nc.dram_tensor("idx_out", [1, k], F32, kind="ExternalOutput")

    with tile.TileContext(nc) as tc:
        with (
            tc.tile_pool(name="const", bufs=1) as cp,
            tc.tile_pool(name="loop", bufs=2) as lp,
        ):
            xyzp = cp.tile([P, 3 * C + 4], F32, tag="xyzp")
            g2 = cp.tile([P, C], F32, tag="g2")
            dists = cp.tile([P, C], F32, tag="dists")
            idxraw = cp.tile([1, k], F32, tag="idxraw")

            nc.sync.dma_start(xyzp[:], xyzp_d[:])
            xs = xyzp[:, 0:C]
            ys = xyzp[:, C : 2 * C]
            zs = xyzp[:, 2 * C : 3 * C]
            pt0 = xyzp[:, 3 * C : 3 * C + 4]
            nc.vector.memset(idxraw[:], BIGI)  # sample 0 is point 0

            # generate flat point index p*C + c on device, then derive the
            # argmax-encoding plane g2 = BIGI - idx and the init distances
            # (+BIG valid / -BIG pad) from it
            ii = cp.tile([P, C], I32, tag="ii")
            nc.gpsimd.iota(ii[:], [[1, C]], channel_multiplier=C)
            idxpc = cp.tile([P, C], F32, tag="idxpc")
            nc.vector.tensor_copy(idxpc[:], ii[:])
            nc.vector.tensor_scalar(
                g2[:], idxpc[:], -1.0, BIGI, op0=Alu.mult, op1=Alu.add
            )
            valid = cp.tile([P, C], F32, tag="valid")
            nc.vector.tensor_scalar(valid[:], idxpc[:], float(n), None, op0=Alu.is_lt)
            nc.vector.tensor_scalar(
                dists[:], valid[:], 2.0 * BIG, -BIG, op0=Alu.mult, op1=Alu.add
            )

            pt = None
            for it in range(k - 1):
                if pt is None:
                    px, py, pz = (
                        xyzp[:, 3 * C + i : 3 * C + i + 1] for i in range(3)
                    )
                else:
                    px = pt[:, 0:1]
                    py = pt[:, 1:2]
                    pz = pt[:, 2:3]
                # d = ((x-px)^2 + (y-py)^2) + (z-pz)^2, bit-exact f32
                t1 = lp.tile([P, C], F32, tag="t1")
                nc.scalar.activation(t1[:], xs, Act.Square, bias=px, scale=-1.0)
                t2 = lp.tile([P, C], F32, tag="t2")
                nc.scalar.activation(t2[:], ys, Act.Square, bias=py, scale=-1.0)
                t3 = lp.tile([P, C], F32, tag="t3")
                nc.scalar.activation(t3[:], zs, Act.Square, bias=pz, scale=-1.0)
                s = lp.tile([P, C], F32, tag="s")
                nc.vector.tensor_tensor(s[:], t1[:], t2[:], op=Alu.add)
                nc.vector.tensor_tensor(s[:], s[:], t3[:], op=Alu.add)
                # dists = min(dists, d); permax = rowwise max of new dists
                # (tensor_tensor_reduce would fuse these but crashes this
                # runtime, so keep them split)
                permax = lp.tile([P, 1], F32, tag="permax")
                nc.vector.tensor_tensor(dists[:], dists[:], s[:], op=Alu.min)
                nc.vector.reduce_max(permax[:], dists[:], axis=mybir.AxisListType.X)
                gmax = lp.tile([P, 1], F32, tag="gmax")
                nc.gpsimd.partition_all_reduce(
                    gmax[:], permax[:], channels=P, reduce_op=bass_isa.ReduceOp.max
                )
                # encode argmax as max over (dists==gmax)*(BIGI-idx)
                mi = lp.tile([P, C], F32, tag="mi")
                nc.vector.scalar_tensor_tensor(
                    mi[:], in0=dists[:], scalar=gmax[:], in1=g2[:],
                    op0=Alu.is_equal, op1=Alu.mult,
                )
                permax2 = lp.tile([P, 1], F32, tag="permax2")
                nc.vector.reduce_max(permax2[:], mi[:], axis=mybir.AxisListType.X)
                is2 = lp.tile([P, 1], F32, tag="is2")
                nc.gpsimd.partition_all_reduce(
                    is2[:], permax2[:], channels=P, reduce_op=bass_isa.ReduceOp.max
                )
                # record BIGI - idx (decoded after the loop)
                nc.scalar.copy(idxraw[0:1, it + 1 : it + 2], is2[0:1, 0:1])
                # extract winner coords: one-hot (g2==is2) dot each plane
                ptn = lp.tile([P, 4], F32, tag="ptn")
                junk = lp.tile([P, C], F32, tag="junk")
                nc.vector.scalar_tensor_tensor(
                    junk[:], in0=g2[:], scalar=is2[:], in1=xs,
                    op0=Alu.is_equal, op1=Alu.mult, accum_out=ptn[:, 0:1],
                )
                nc.vector.scalar_tensor_tensor(
                    junk[:], in0=g2[:], scalar=is2[:], in1=ys,
                    op0=Alu.is_equal, op1=Alu.mult, accum_out=ptn[:, 1:2],
                )
                nc.vector.scalar_tensor_tensor(
                    junk[:], in0=g2[:], scalar=is2[:], in1=zs,
                    op0=Alu.is_equal, op1=Alu.mult, accum_out=ptn[:, 2:3],
                )
                ptb = lp.tile([P, 4], F32, tag="ptb")
                nc.gpsimd.partition_all_reduce(
                    ptb[:, 0:3], ptn[:, 0:3], channels=P,
                    reduce_op=bass_isa.ReduceOp.add,
                )
                pt = ptb

            # decode indices: idx = BIGI - idxraw
            idxf = cp.tile([1, k], F32, tag="idxf")
            nc.vector.tensor_scalar(
                idxf[:], idxraw[:], -1.0, BIGI, op0=Alu.mult, op1=Alu.add
            )
            nc.sync.dma_start(idx_d[:], idxf[:])

    nc.compile()
    return nc


def build_linear_kernel(k=K, d_in=D_IN, d_out=D_OUT):
    """Phase-2 program: out = rowsT.T @ W + b for one batch element."""
    assert k % P == 0 and d_in == P
    kg = k // P

    nc = bacc.Bacc("TRN2", target_bir_lowering=False)

    # the whole linear phase runs in bf16 (f32 PSUM accumulation): halves
    # both the rowsT upload and the result download.  Combined with the
    # bf16 result rounding this costs ~5e-3 relative error against the
    # 2e-2 harness tolerance.
    BF16 = mybir.dt.bfloat16
    rowsT_d = nc.dram_tensor("rowsT", [d_in, k], BF16, kind="ExternalInput")
    w_d = nc.dram_tensor("w", [d_in, d_out], BF16, kind="ExternalInput")
    brow_d = nc.dram_tensor("brow", [1, d_out], BF16, kind="ExternalInput")
    out_d = nc.dram_tensor("out", [k, d_out], BF16, kind="ExternalOutput")

    with tile.TileContext(nc) as tc:
        with (
            tc.tile_pool(name="const", bufs=1) as cp,
            tc.tile_pool(name="psum", bufs=2, space="PSUM") as pp,
        ):
            rowsT = cp.tile([d_in, k], BF16, tag="rowsT")
            w_sb = cp.tile([d_in, d_out], BF16, tag="w")
            brow = cp.tile([1, d_out], BF16, tag="brow")
            ones1 = cp.tile([1, P], BF16, tag="ones1")
            nc.sync.dma_start(rowsT[:], rowsT_d[:])
            nc.sync.dma_start(w_sb[:], w_d[:])
            nc.sync.dma_start(brow[:], brow_d[:])
            nc.vector.memset(ones1[:], 1.0)

            for j in range(kg):
                out_ps = pp.tile([P, d_out], F32, tag="outps")
                nc.tensor.matmul(
                    out_ps[:], lhsT=rowsT[:, j * P : (j + 1) * P], rhs=w_sb[:],
                    start=True, stop=False,
                )
                nc.tensor.matmul(
                    out_ps[:], lhsT=ones1[:], rhs=brow[:], start=False, stop=True
                )
                outt = cp.tile([P, d_out], BF16, tag=f"outt{j}")
                nc.vector.tensor_copy(outt[:], out_ps[:])
                nc.sync.dma_start(out_d[j * P : (j + 1) * P, :], outt[:])

    nc.compile()
    return nc


def fill_fps_inputs(xyzp, means_b, n=N):
    """Pack one batch element's coordinate planes into a [P, 3C+4] view."""
    C = _ceil_div(n, P)
    npad = P * C
    m = np.asarray(means_b, np.float32)
    planes = np.zeros((npad, 3), np.float32)
    planes[:n] = m
    for i in range(3):
        xyzp[:, i * C : (i + 1) * C] = planes[:, i].reshape(P, C)
    xyzp[:, 3 * C : 3 * C + 3] = m[0]
    xyzp[:, 3 * C + 3] = 0.0


_CACHE = {}


def _make_dispatcher(nc, n_cores):
    """Build the PJRT dispatch closure ONCE per program.

    This replicates run_bass_kernel_spmd's axon path (bass2jax.
    run_bass_via_pjrt) but hoists the jax.jit(shard_map(...)) out of the
    per-call path: run_bass_kernel_spmd constructs a fresh jit closure
    every call, which re-lowers the XLA module and re-runs neuronx_cc_hook
    -> compile_bir_kernel (~0.5-1s of BIR re-verification per call even
    with a warm backend).  Holding one jitted callable hits jax's cpp-jit
    fast path on repeat calls, leaving only input transfer + execution.
    """
    import jax
    from jax.experimental.shard_map import shard_map
    from jax.sharding import Mesh, PartitionSpec

    from concourse import bass2jax
    from concourse.bass2jax import _bass_exec_p, install_neuronx_cc_hook

    install_neuronx_cc_hook()

    partition_name = (
        nc.partition_id_tensor.name if nc.partition_id_tensor is not None else None
    )
    dbg_name = nc.dbg_addr.name if nc.dbg_addr is not None else None
    if dbg_name is not None:
        assert not nc.dbg_callbacks

    in_names, out_names, out_avals = [], [], []
    for alloc in nc.m.functions[0].allocations:
        if not isinstance(alloc, mybir.MemoryLocationSet):
            continue
        name = alloc.memorylocations[0].name
        if alloc.kind == "ExternalInput":
            if name != partition_name:
                in_names.append(name)
        elif alloc.kind == "ExternalOutput":
            out_names.append(name)
            out_avals.append(
                jax.core.ShapedArray(
                    tuple(alloc.tensor_shape), mybir.dt.np(alloc.dtype)
                )
            )
    n_params = len(in_names)
    bind_in_names = list(in_names) + list(out_names)
    if partition_name is not None:
        bind_in_names.append(partition_name)

    def _body(*args):
        operands = list(args)
        if partition_name is not None:
            operands.append(bass2jax.partition_id_tensor())
        outs = _bass_exec_p.bind(
            *operands,
            out_avals=tuple(out_avals),
            in_names=tuple(bind_in_names),
            out_names=tuple(out_names),
            lowering_input_output_aliases=(),
            sim_require_finite=True,
            sim_require_nnan=True,
            nc=nc,
        )
        return tuple(outs)

    devices = jax.devices()[:n_cores]
    mesh = Mesh(np.asarray(devices), ("core",))
    sharded = jax.jit(
        shard_map(
            _body,
            mesh=mesh,
            in_specs=(PartitionSpec("core"),) * (n_params + len(out_names)),
            out_specs=(PartitionSpec("core"),) * len(out_names),
            check_rep=False,
        ),
        keep_unused=True,
    )

    # The "pre-zeroed output" operands run_bass_via_pjrt ships from host
    # every call are never read back by these programs (every output element
    # is written), and without donation the buffers survive the call — so
    # place them on device once and reuse them.
    from jax.sharding import NamedSharding

    zero_args = [
        jax.device_put(
            np.zeros((n_cores * a.shape[0], *a.shape[1:]), a.dtype),
            NamedSharding(mesh, PartitionSpec("core")),
        )
        for a in out_avals
    ]

    def dispatch(in_maps=None, preplaced=None, concat=None):
        if dbg_name is not None and in_maps is not None:
            in_maps = [
                {**m, dbg_name: np.zeros((1, 2), np.uint32)} for m in in_maps
            ]

        def _arg(name):
            if preplaced is not None and name in preplaced:
                return preplaced[name]
            if concat is not None and name in concat:
                return concat[name]
            if name == dbg_name and in_maps is None:
                return np.zeros((n_cores, 2), np.uint32)
            return np.concatenate([np.asarray(m[name]) for m in in_maps], axis=0)

        out_arrs = sharded(*[_arg(name) for name in in_names], *zero_args)
        return [
            {
                name: np.asarray(out_arrs[i]).reshape(
                    n_cores, *out_avals[i].shape
                )[c]
                for i, name in enumerate(out_names)
            }
            for c in range(n_cores)
        ]

    dispatch.put = lambda arr: jax.device_put(
        arr, NamedSharding(mesh, PartitionSpec("core"))
    )
    return dispatch


def _get_kernels():
    if "fps_run" not in _CACHE:
        _CACHE["fps_run"] = _make_dispatcher(build_fps_kernel(), B)
        _CACHE["lin_run"] = _make_dispatcher(build_linear_kernel(), B)
    return _CACHE["fps_run"], _CACHE["lin_run"]


def kernel(features, means, W, b, trace=False):
    features = np.asarray(features, np.float32)
    means = np.asarray(means, np.float32)
    W = np.ascontiguousarray(W, np.float32)
    brow = np.ascontiguousarray(b, np.float32).reshape(1, -1)

    fps_run, lin_run = _get_kernels()
    import time as _time

    C = _ceil_div(N, P)
    t0 = _time.time()
    xyzp_all = np.empty((B * P, 3 * C + 4), np.float32)
    for bb in range(B):
        fill_fps_inputs(xyzp_all[bb * P : (bb + 1) * P], means[bb])
    t1 = _time.time()
    res1 = fps_run(concat={"xyzp": xyzp_all})
    t2 = _time.time()
    idx = np.stack(
        [np.rint(res1[bb]["idx_out"][0]).astype(np.int64) for bb in range(B)]
    )  # [B, K]

    # model weights are loaded to device once and kept resident (re-uploaded
    # only if the caller passes different weights)
    import ml_dtypes

    bf16 = ml_dtypes.bfloat16
    if _CACHE.get("w_host") is None or not (
        np.array_equal(W, _CACHE["w_host"])
        and np.array_equal(brow, _CACHE["b_host"])
    ):
        _CACHE["w_host"] = W.copy()
        _CACHE["b_host"] = brow.copy()
        _CACHE["w_dev"] = lin_run.put(
            np.concatenate([W] * B, axis=0).astype(bf16)
        )
        _CACHE["b_dev"] = lin_run.put(
            np.concatenate([brow] * B, axis=0).astype(bf16)
        )

    rowsT_all = np.empty((B * D_IN, K), bf16)
    for bb in range(B):
        rowsT_all[bb * D_IN : (bb + 1) * D_IN] = features[bb][idx[bb]].T
    t3 = _time.time()
    res2 = lin_run(
        concat={"rowsT": rowsT_all},
        preplaced={"w": _CACHE["w_dev"], "brow": _CACHE["b_dev"]},
    )
    t4 = _time.time()
    _CACHE["phase_s"] = (t1 - t0, t2 - t1, t3 - t2, t4 - t3)
    _CACHE["last_run_s"] = t4 - t0
    out = np.stack(
        [res2[bb]["out"].astype(np.float32) for bb in range(B)], axis=0
    )
    return out


if __name__ == "__main__":
    ins = dict(np.load("/tmp/inputs.npz"))
    out = kernel(**ins)
    print("out", out.shape, out.dtype)


# revision 30
# speedup vs baseline: 61.9369x; 1.1117x over previous
"""Trainium2 Bass kernel for GaussianFPSPooling.

The axon tunnel to the device moves ~48 MB/s H2D / ~28 MB/s D2H on one
serialized stream (transfers to different cores do NOT parallelize), so
the run is dominated by host<->device transfer, not compute.  The old
baseline shipped the full features tensor (8 x 51.2 MB) every call and
took ~7-10 s.  This version never sends features to the device:

  Phase 1 (device, cores 0-3, one batch element per core):
      Farthest-point sampling over N=100000 3-D points, K=256 iterations,
      fully SBUF-resident.  Arithmetic replicates the jax-CPU reference
      bit-exactly ((x-px)^2 + (y-py)^2) + (z-pz)^2, f32, left-assoc, min
      accumulate, first-index argmax) so the selected indices match
      exactly.  Only the coordinate planes go over the wire (4.8 MB
      total, packed into ONE input tensor per core = one transfer RPC);
      the index-encoding plane and the init-distance plane are generated
      on-device with iota.  Returns the K indices (1 KB/core).
      Predicted device exec: ~2.9 ms.

  Host: gather the K=256 selected feature rows per batch (numpy fancy
      indexing, ~1 ms) and transpose to [d_in, K].

  Phase 2 (device, core 0, all batches): rowsT.T @ W + b via 8 PE
      matmul pairs (+ ones x b trick for the bias), all operands bf16
      with f32 PSUM accumulation; the result leaves the device in bf16
      and the host upcasts.  W/b are placed on device once and kept
      resident across calls (standard weight residency), so the
      steady-state wire cost is rowsT in (256 KB) + out back (512 KB).
      (Splitting either phase into more, smaller dispatches was tried
      and is a big loss: a dispatch round trip over the tunnel costs
      ~60 ms unless amortized against a large in-flight payload.)

Dispatch goes through a jax.jit(shard_map(bass_exec)) callable that is
built ONCE per program and cached: run_bass_kernel_spmd rebuilds the jit
closure every call, which re-runs the XLA->neuronx lowering hook and
~0.5-1 s of BIR re-verification per call.

Wire traffic ~5.6 MB/call instead of ~410 MB; steady-state wall time
~0.16-0.18 s/call vs 6.9-10 s for the baseline (~40-60x).
"""

import sys

if "/opt/trn_rl_repo" not in sys.path:
    sys.path.insert(0, "/opt/trn_rl_repo")

import numpy as np

import concourse.bacc as bacc
import concourse.bass_isa as bass_isa
import concourse.mybir as mybir
from concourse import tile

F32 = mybir.dt.float32
I32 = mybir.dt.int32
Alu = mybir.AluOpType
Act = mybir.ActivationFunctionType

# problem sizes (hardcoded per contract)
B = 4
N = 100000
D_IN = 128
D_OUT = 256
K = 256
P = 128               # partitions
BIGI = float(1 << 20)  # index-encoding base: stores BIGI - idx (exact in f32)
BIG = 1.0e30          # init "infinity" for valid entries; pad gets -BIG


def _ceil_div(a, b):
    return (a + b - 1) // b


def build_fps_kernel(n=N, k=K):
    """Phase-1 program: FPS over one batch element; emits the K indices."""
    C = _ceil_div(n, P)

    nc = bacc.Bacc("TRN2", target_bir_lowering=False)

    # all coordinate planes + the seed point packed into ONE input tensor:
    # a single transfer RPC over the (high-latency) axon link
    xyzp_d = nc.dram_tensor("xyzp", [P, 3 * C + 4], F32, kind="ExternalInput")
    idx_d = nc.dram_tensor("idx_out", [1, k], F32, kind="ExternalOutput")

    with tile.TileContext(nc) as tc:
        with (
            tc.tile_pool(name="const", bufs=1) as cp,
            tc.tile_pool(name="loop", bufs=2) as lp,
        ):
            xyzp = cp.tile([P, 3 * C + 4], F32, tag="xyzp")
            g2 = cp.tile([P, C], F32, tag="g2")
            dists = cp.tile([P, C], F32, tag="dists")
            idxraw = cp.tile([1, k], F32, tag="idxraw")

            nc.sync.dma_start(xyzp[:], xyzp_d[:])
            xs = xyzp[:, 0:C]
            ys = xyzp[:, C : 2 * C]
            zs = xyzp[:, 2 * C : 3 * C]
            pt0 = xyzp[:, 3 * C : 3 * C + 4]
            nc.vector.memset(idxraw[:], BIGI)  # sample 0 is point 0

            # generate flat point index p*C + c on device, then derive the
            # argmax-encoding plane g2 = BIGI - idx and the init distances
            # (+BIG valid / -BIG pad) from it
            ii = cp.tile([P, C], I32, tag="ii")
            nc.gpsimd.iota(ii[:], [[1, C]], channel_multiplier=C)
            idxpc = cp.tile([P, C], F32, tag="idxpc")
            nc.vector.tensor_copy(idxpc[:], ii[:])
            nc.vector.tensor_scalar(
                g2[:], idxpc[:], -1.0, BIGI, op0=Alu.mult, op1=Alu.add
            )
            valid = cp.tile([P, C], F32, tag="valid")
            nc.vector.tensor_scalar(valid[:], idxpc[:], float(n), None, op0=Alu.is_lt)
            nc.vector.tensor_scalar(
                dists[:], valid[:], 2.0 * BIG, -BIG, op0=Alu.mult, op1=Alu.add
            )

            pt = None
            for it in range(k - 1):
                if pt is None:
                    px, py, pz = (
                        xyzp[:, 3 * C + i : 3 * C + i + 1] for i in range(3)
                    )
                else:
                    px = pt[:, 0:1]
                    py = pt[:, 1:2]
                    pz = pt[:, 2:3]
                # d = ((x-px)^2 + (y-py)^2) + (z-pz)^2, bit-exact f32
                t1 = lp.tile([P, C], F32, tag="t1")
                nc.scalar.activation(t1[:], xs, Act.Square, bias=px, scale=-1.0)
                t2 = lp.tile([P, C], F32, tag="t2")
                nc.scalar.activation(t2[:], ys, Act.Square, bias=py, scale=-1.0)
                t3 = lp.tile([P, C], F32, tag="t3")
                nc.scalar.activation(t3[:], zs, Act.Square, bias=pz, scale=-1.0)
                s = lp.tile([P, C], F32, tag="s")
                nc.vector.tensor_tensor(s[:], t1[:], t2[:], op=Alu.add)
                nc.vector.tensor_tensor(s[:], s[:], t3[:], op=Alu.add)
                # dists = min(dists, d); permax = rowwise max of new dists
                # (tensor_tensor_reduce would fuse these but crashes this
                # runtime, so keep them split)
                permax = lp.tile([P, 1], F32, tag="permax")
                nc.vector.tensor_tensor(dists[:], dists[:], s[:], op=Alu.min)
                nc.vector.reduce_max(permax[:], dists[:], axis=mybir.AxisListType.X)
                gmax = lp.tile([P, 1], F32, tag="gmax")
                nc.gpsimd.partition_all_reduce(
                    gmax[:], permax[:], channels=P, reduce_op=bass_isa.ReduceOp.max
                )
                # encode argmax as max over (dists==gmax)*(BIGI-idx)
                mi = lp.tile([P, C], F32, tag="mi")
                nc.vector.scalar_tensor_tensor(
                    mi[:], in0=dists[:], scalar=gmax[:], in1=g2[:],
                    op0=Alu.is_equal, op1=Alu.mult,
                )
                permax2 = lp.tile([P, 1], F32, tag="permax2")
                nc.vector.reduce_max(permax2[:], mi[:], axis=mybir.AxisListType.X)
                is2 = lp.tile([P, 1], F32, tag="is2")
                nc.gpsimd.partition_all_reduce(
                    is2[:], permax2[:], channels=P, reduce_op=bass_isa.ReduceOp.max
                )
                # record BIGI - idx (decoded after the loop)
                nc.scalar.copy(idxraw[0:1, it + 1 : it + 2], is2[0:1, 0:1])
                # extract winner coords: one-hot (g2==is2) dot each plane
                ptn = lp.tile([P, 4], F32, tag="ptn")
                junk = lp.tile([P, C], F32, tag="junk")
                nc.vector.scalar_tensor_tensor(
                    junk[:], in0=g2[:], scalar=is2[:], in1=xs,
                    op0=Alu.is_equal, op1=Alu.mult, accum_out=ptn[:, 0:1],
                )
                nc.vector.scalar_tensor_tensor(
                    junk[:], in0=g2[:], scalar=is2[:], in1=ys,
                    op0=Alu.is_equal, op1=Alu.mult, accum_out=ptn[:, 1:2],
                )
                nc.vector.scalar_tensor_tensor(
                    junk[:], in0=g2[:], scalar=is2[:], in1=zs,
                    op0=Alu.is_equal, op1=Alu.mult, accum_out=ptn[:, 2:3],
                )
                ptb = lp.tile([P, 4], F32, tag="ptb")
                nc.gpsimd.partition_all_reduce(
                    ptb[:, 0:3], ptn[:, 0:3], channels=P,
                    reduce_op=bass_isa.ReduceOp.add,
                )
                pt = ptb

            # decode indices: idx = BIGI - idxraw
            idxf = cp.tile([1, k], F32, tag="idxf")
            nc.vector.tensor_scalar(
                idxf[:], idxraw[:], -1.0, BIGI, op0=Alu.mult, op1=Alu.add
            )
            nc.sync.dma_start(idx_d[:], idxf[:])

    nc.compile()
    return nc


def build_linear_kernel(k=B * K, d_in=D_IN, d_out=D_OUT):
    """Phase-2 program: out = rowsT.T @ W + b, all batches on one core.

    A single core keeps every transfer single-shard (the axon link adds
    per-shard RPC latency); the 8 matmul pairs are still negligible work.
    """
    assert k % P == 0 and d_in == P
    kg = k // P

    nc = bacc.Bacc("TRN2", target_bir_lowering=False)

    # the whole linear phase runs in bf16 (f32 PSUM accumulation): halves
    # both the rowsT upload and the result download.  Combined with the
    # bf16 result rounding this costs ~5e-3 relative error against the
    # 2e-2 harness tolerance.
    BF16 = mybir.dt.bfloat16
    rowsT_d = nc.dram_tensor("rowsT", [d_in, k], BF16, kind="ExternalInput")
    w_d = nc.dram_tensor("w", [d_in, d_out], BF16, kind="ExternalInput")
    brow_d = nc.dram_tensor("brow", [1, d_out], BF16, kind="ExternalInput")
    out_d = nc.dram_tensor("out", [k, d_out], BF16, kind="ExternalOutput")

    with tile.TileContext(nc) as tc:
        with (
            tc.tile_pool(name="const", bufs=1) as cp,
            tc.tile_pool(name="psum", bufs=2, space="PSUM") as pp,
        ):
            rowsT = cp.tile([d_in, k], BF16, tag="rowsT")
            w_sb = cp.tile([d_in, d_out], BF16, tag="w")
            brow = cp.tile([1, d_out], BF16, tag="brow")
            ones1 = cp.tile([1, P], BF16, tag="ones1")
            nc.sync.dma_start(rowsT[:], rowsT_d[:])
            nc.sync.dma_start(w_sb[:], w_d[:])
            nc.sync.dma_start(brow[:], brow_d[:])
            nc.vector.memset(ones1[:], 1.0)

            for j in range(kg):
                out_ps = pp.tile([P, d_out], F32, tag="outps")
                nc.tensor.matmul(
                    out_ps[:], lhsT=rowsT[:, j * P : (j + 1) * P], rhs=w_sb[:],
                    start=True, stop=False,
                )
                nc.tensor.matmul(
                    out_ps[:], lhsT=ones1[:], rhs=brow[:], start=False, stop=True
                )
                outt = cp.tile([P, d_out], BF16, tag=f"outt{j}")
                nc.vector.tensor_copy(outt[:], out_ps[:])
                nc.sync.dma_start(out_d[j * P : (j + 1) * P, :], outt[:])

    nc.compile()
    return nc


def fill_fps_inputs(xyzp, means_b, n=N):
    """Pack one batch element's coordinate planes into a [P, 3C+4] view."""
    C = _ceil_div(n, P)
    npad = P * C
    m = np.asarray(means_b, np.float32)
    planes = np.zeros((npad, 3), np.float32)
    planes[:n] = m
    for i in range(3):
        xyzp[:, i * C : (i + 1) * C] = planes[:, i].reshape(P, C)
    xyzp[:, 3 * C : 3 * C + 3] = m[0]
    xyzp[:, 3 * C + 3] = 0.0


_CACHE = {}


def _make_dispatcher(nc, n_cores):
    """Build the PJRT dispatch closure ONCE per program.

    This replicates run_bass_kernel_spmd's axon path (bass2jax.
    run_bass_via_pjrt) but hoists the jax.jit(shard_map(...)) out of the
    per-call path: run_bass_kernel_spmd constructs a fresh jit closure
    every call, which re-lowers the XLA module and re-runs neuronx_cc_hook
    -> compile_bir_kernel (~0.5-1s of BIR re-verification per call even
    with a warm backend).  Holding one jitted callable hits jax's cpp-jit
    fast path on repeat calls, leaving only input transfer + execution.
    """
    import jax
    from jax.experimental.shard_map import shard_map
    from jax.sharding import Mesh, PartitionSpec

    from concourse import bass2jax
    from concourse.bass2jax import _bass_exec_p, install_neuronx_cc_hook

    install_neuronx_cc_hook()

    partition_name = (
        nc.partition_id_tensor.name if nc.partition_id_tensor is not None else None
    )
    dbg_name = nc.dbg_addr.name if nc.dbg_addr is not None else None
    if dbg_name is not None:
        assert not nc.dbg_callbacks

    in_names, out_names, out_avals = [], [], []
    for alloc in nc.m.functions[0].allocations:
        if not isinstance(alloc, mybir.MemoryLocationSet):
            continue
        name = alloc.memorylocations[0].name
        if alloc.kind == "ExternalInput":
            if name != partition_name:
                in_names.append(name)
        elif alloc.kind == "ExternalOutput":
            out_names.append(name)
            out_avals.append(
                jax.core.ShapedArray(
                    tuple(alloc.tensor_shape), mybir.dt.np(alloc.dtype)
                )
            )
    n_params = len(in_names)
    bind_in_names = list(in_names) + list(out_names)
    if partition_name is not None:
        bind_in_names.append(partition_name)

    def _body(*args):
        operands = list(args)
        if partition_name is not None:
            operands.append(bass2jax.partition_id_tensor())
        outs = _bass_exec_p.bind(
            *operands,
            out_avals=tuple(out_avals),
            in_names=tuple(bind_in_names),
            out_names=tuple(out_names),
            lowering_input_output_aliases=(),
            sim_require_finite=True,
            sim_require_nnan=True,
            nc=nc,
        )
        return tuple(outs)

    devices = jax.devices()[:n_cores]
    mesh = Mesh(np.asarray(devices), ("core",))
    sharded = jax.jit(
        shard_map(
            _body,
            mesh=mesh,
            in_specs=(PartitionSpec("core"),) * (n_params + len(out_names)),
            out_specs=(PartitionSpec("core"),) * len(out_names),
            check_rep=False,
        ),
        keep_unused=True,
    )

    # The "pre-zeroed output" operands run_bass_via_pjrt ships from host
    # every call are never read back by these programs (every output element
    # is written), and without donation the buffers survive the call — so
    # place them on device once and reuse them.
    from jax.sharding import NamedSharding

    zero_args = [
        jax.device_put(
            np.zeros((n_cores * a.shape[0], *a.shape[1:]), a.dtype),
            NamedSharding(mesh, PartitionSpec("core")),
        )
        for a in out_avals
    ]

    def dispatch(in_maps=None, preplaced=None, concat=None):
        if dbg_name is not None and in_maps is not None:
            in_maps = [
                {**m, dbg_name: np.zeros((1, 2), np.uint32)} for m in in_maps
            ]

        def _arg(name):
            if preplaced is not None and name in preplaced:
                return preplaced[name]
            if concat is not None and name in concat:
                return concat[name]
            if name == dbg_name and in_maps is None:
                return np.zeros((n_cores, 2), np.uint32)
            return np.concatenate([np.asarray(m[name]) for m in in_maps], axis=0)

        out_arrs = sharded(*[_arg(name) for name in in_names], *zero_args)
        return [
            {
                name: np.asarray(out_arrs[i]).reshape(
                    n_cores, *out_avals[i].shape
                )[c]
                for i, name in enumerate(out_names)
            }
            for c in range(n_cores)
        ]

    dispatch.put = lambda arr: jax.device_put(
        arr, NamedSharding(mesh, PartitionSpec("core"))
    )
    return dispatch


def _get_kernels():
    if "fps_run" not in _CACHE:
        _CACHE["fps_run"] = _make_dispatcher(build_fps_kernel(), B)
        _CACHE["lin_run"] = _make_dispatcher(build_linear_kernel(), 1)
    return _CACHE["fps_run"], _CACHE["lin_run"]


def kernel(features, means, W, b, trace=False):
    features = np.asarray(features, np.float32)
    means = np.asarray(means, np.float32)
    W = np.ascontiguousarray(W, np.float32)
    brow = np.ascontiguousarray(b, np.float32).reshape(1, -1)

    fps_run, lin_run = _get_kernels()
    import time as _time

    C = _ceil_div(N, P)
    t0 = _time.time()
    xyzp_all = np.empty((B * P, 3 * C + 4), np.float32)
    for bb in range(B):
        fill_fps_inputs(xyzp_all[bb * P : (bb + 1) * P], means[bb])
    t1 = _time.time()
    res1 = fps_run(concat={"xyzp": xyzp_all})
    t2 = _time.time()
    idx = np.stack(
        [np.rint(res1[bb]["idx_out"][0]).astype(np.int64) for bb in range(B)]
    )  # [B, K]

    # model weights are loaded to device once and kept resident (re-uploaded
    # only if the caller passes different weights)
    import ml_dtypes

    bf16 = ml_dtypes.bfloat16
    if _CACHE.get("w_host") is None or not (
        np.array_equal(W, _CACHE["w_host"])
        and np.array_equal(brow, _CACHE["b_host"])
    ):
        _CACHE["w_host"] = W.copy()
        _CACHE["b_host"] = brow.copy()
        _CACHE["w_dev"] = lin_run.put(W.astype(bf16))
        _CACHE["b_dev"] = lin_run.put(brow.astype(bf16))

    rowsT_all = np.empty((D_IN, B * K), bf16)
    for bb in range(B):
        rowsT_all[:, bb * K : (bb + 1) * K] = features[bb][idx[bb]].T
    t3 = _time.time()
    res2 = lin_run(
        concat={"rowsT": rowsT_all},
        preplaced={"w": _CACHE["w_dev"], "brow": _CACHE["b_dev"]},
    )
    t4 = _time.time()
    _CACHE["phase_s"] = (t1 - t0, t2 - t1, t3 - t2, t4 - t3)
    _CACHE["last_run_s"] = t4 - t0
    out = (
        res2[0]["out"].astype(np.float32).reshape(B, K, D_OUT)
    )
    return out


if __name__ == "__main__":
    ins = dict(np.load("/tmp/inputs.npz"))
    out = kernel(**ins)
    print("out", out.shape, out.dtype)
